# revision 1
# baseline (speedup 1.0000x reference)
"""3-layer GCN (PyG GCNConv-style) forward on 8 Trainium2 NeuronCores.

Self-contained harness entry: kernel(**inputs) -> np.ndarray [100000, 128].

Strategy (graph/data parallel, dst-sharded):
  - nodes sharded contiguously across 8 cores (12500/core)
  - per layer: each core computes g = (drop(h) @ W) * dinv for its shard
    (dropout mask and src-side dinv are folded into one multiplier on the
    host), AllGather g into a replicated HBM table, then each core runs
    gather / scatter-add over its incoming edges with the gpsimd MoE DMA
    primitives (dma_gather / dma_scatter_add), then a readback pass applies
    the dst-side dinv, bias, relu and transposes for the next layer.
  - scatter-add calls must have unique slots (the HW CCE RMW races on
    duplicate addresses within one call): edges are dealt into occurrence
    groups of 2; slot = occ_parity*NSP + dst. Group parity alternates
    between two accumulator tensors so consecutive scatter calls overlap.
  - all cores share one SPMD NEFF: per-(block, group) segment sizes are
    padded to the max across cores; pads gather row 0 and scatter into a
    trash region with in-call-unique slots.
"""

import numpy as np
from dataclasses import dataclass, field

N = 100000
E = 1600000
D = 128
P = 8


@dataclass
class Cfg:
    N: int = N
    E: int = E
    D: int = D
    P: int = P
    CH: int = 4096          # max idxs per gather/scatter call
    TRASH: int = 4096       # trash slot region (>= CH)
    reps: int = 1           # body replications (for timing harnesses)

    @property
    def NS(self):
        return self.N // self.P

    @property
    def NSP(self):
        return (self.NS + 127) // 128 * 128

    @property
    def NT(self):
        return self.NSP // 128

    @property
    def NBLK(self):
        return self.P // 2

    @property
    def BROWS(self):
        return 2 * self.NSP

    @property
    def TROWS(self):
        return self.P * self.NSP

    @property
    def ACCR(self):
        return 2 * self.NSP + self.TRASH


@dataclass
class Template:
    chunks: list = field(default_factory=list)
    TOT: int = 0


def build_template(cfg, seg_sizes):
    tpl = Template()
    pos = 0
    for b in range(cfg.NBLK):
        gs = sorted(g for (bb, g) in seg_sizes if bb == b)
        pieces = []
        for g in gs:
            S = (seg_sizes[(b, g)] + 127) // 128 * 128
            off = 0
            while off < S:
                ln = min(cfg.CH, S - off)
                pieces.append((g % 2, pos + off, pos + off + ln))
                off += ln
            pos += S
        cur, cur_len = [], 0
        for pc in pieces:
            ln = pc[2] - pc[1]
            if cur and cur_len + ln > cfg.CH:
                tpl.chunks.append((b, cur[0][1], cur[-1][2], cur))
                cur, cur_len = [], 0
            cur.append(pc)
            cur_len += ln
        if cur:
            tpl.chunks.append((b, cur[0][1], cur[-1][2], cur))
    tpl.TOT = pos
    return tpl


def prep_edges(cfg, src, dst):
    NS, NSP, CH = cfg.NS, cfg.NSP, cfg.CH
    per_core = []
    seg_max = {}
    core_segs = []
    for c in range(cfg.P):
        sel = (dst >= c * NS) & (dst < (c + 1) * NS)
        es = src[sel].astype(np.int64)
        ed = (dst[sel] - c * NS).astype(np.int64)
        order = np.argsort(ed, kind="stable")
        es, ed = es[order], ed[order]
        if len(ed):
            first = np.r_[True, ed[1:] != ed[:-1]]
            run_start = np.maximum.accumulate(
                np.where(first, np.arange(len(ed)), 0))
            m = np.arange(len(ed)) - run_start
        else:
            m = np.zeros(0, np.int64)
        g = m // 2
        slot = (m % 2) * NSP + ed
        b = es // (2 * NS)
        src_loc = (es // NS % 2) * NSP + es % NS
        segs = {}
        key = b * 100000 + g
        order2 = np.argsort(key, kind="stable")
        es2 = src_loc[order2]
        sl2 = slot[order2]
        uniq, counts = np.unique(key[order2], return_counts=True)
        off = 0
        for k, cnt in zip(uniq, counts):
            segs[(int(k) // 100000, int(k) % 100000)] = (off, int(cnt))
            off += int(cnt)
        core_segs.append((es2, sl2, segs))
        for kk, (o, cnt) in segs.items():
            seg_max[kk] = max(seg_max.get(kk, 0), cnt)

    tpl = build_template(cfg, seg_max)
    TOT = tpl.TOT
    TRASH0 = 2 * NSP

    for c in range(cfg.P):
        es2, sl2, segs = core_segs[c]
        sarr = np.zeros(TOT, np.int64)
        darr = np.zeros(TOT, np.int64)
        pos = 0
        for b in range(cfg.NBLK):
            gs = sorted(g for (bb, g) in seg_max if bb == b)
            for g in gs:
                S = (seg_max[(b, g)] + 127) // 128 * 128
                o, cnt = segs.get((b, g), (0, 0))
                sarr[pos:pos + cnt] = es2[o:o + cnt]
                darr[pos:pos + cnt] = sl2[o:o + cnt]
                if S - cnt:
                    p = np.arange(pos + cnt, pos + S)
                    darr[pos + cnt:pos + S] = TRASH0 + ((p - pos) % CH)
                pos += S
        sw = np.tile(sarr.reshape(-1, 16).T.astype(np.int16), (8, 1))
        dw = np.tile(darr.reshape(-1, 16).T.astype(np.int16), (8, 1))
        per_core.append((sw, dw))
    return tpl, per_core


def build_nc(cfg, tpl):
    import concourse.bacc as bacc
    import concourse.tile as tile
    from concourse import mybir

    f32 = mybir.dt.float32
    i16 = mybir.dt.int16
    NS, NSP, NT, D = cfg.NS, cfg.NSP, cfg.NT, cfg.D
    TOT = tpl.TOT

    nc = bacc.Bacc("TRN2", target_bir_lowering=False, debug=False,
                   num_devices=cfg.P, num_swdge_queues=1)

    x0T_in = nc.dram_tensor("x0T", [128, NSP], f32, kind="ExternalInput")
    m1T_in = nc.dram_tensor("m1T", [NT, 128, 128], f32, kind="ExternalInput")
    m2T_in = nc.dram_tensor("m2T", [NT, 128, 128], f32, kind="ExternalInput")
    dinv_in = nc.dram_tensor("dinvc", [128, NT], f32, kind="ExternalInput")
    W_in = [nc.dram_tensor(f"W{i}", [128, 128], f32, kind="ExternalInput")
            for i in range(3)]
    bB_in = [nc.dram_tensor(f"bB{i}", [128, 128], f32, kind="ExternalInput")
             for i in range(3)]
    ident_in = nc.dram_tensor("ident", [128, 128], f32, kind="ExternalInput")
    srcw_in = nc.dram_tensor("srcw", [128, TOT // 16], i16, kind="ExternalInput")
    dstw_in = nc.dram_tensor("dstw", [128, TOT // 16], i16, kind="ExternalInput")
    y_out = nc.dram_tensor("y", [NS, D], f32, kind="ExternalOutput")

    gc = [nc.dram_tensor(f"gc{l}", [NSP, D], f32) for l in range(3)]
    table = [nc.dram_tensor(f"table{l}", [cfg.TROWS, D], f32,
                            addr_space="Shared") for l in range(3)]
    accA = [nc.dram_tensor(f"accA{l}", [cfg.ACCR, D], f32) for l in range(3)]
    accB = [nc.dram_tensor(f"accB{l}", [cfg.ACCR, D], f32) for l in range(3)]

    with tile.TileContext(nc) as tc:
        hT = nc.alloc_sbuf_tensor("hT", [128, NSP], f32)
        with tc.tile_pool(name="const", bufs=1) as constp, \
             tc.tile_pool(name="msgp", bufs=3) as msgp, \
             tc.tile_pool(name="idxp", bufs=3) as idxp, \
             tc.tile_pool(name="rb", bufs=4) as rbp, \
             tc.tile_pool(name="gsb", bufs=4) as gsbp, \
             tc.tile_pool(name="mmp", bufs=4, space="PSUM") as mmp:

            Wt, bBt = [], []
            for i in range(3):
                w = constp.tile([128, 128], f32, tag=f"W{i}")
                nc.sync.dma_start(out=w[:], in_=W_in[i][:])
                Wt.append(w)
                bb = constp.tile([128, 128], f32, tag=f"bB{i}")
                nc.sync.dma_start(out=bb[:], in_=bB_in[i][:])
                bBt.append(bb)
            dinvc = constp.tile([128, NT], f32, tag="dinvc")
            nc.sync.dma_start(out=dinvc[:], in_=dinv_in[:])
            ident = constp.tile([128, 128], f32, tag="ident")
            nc.sync.dma_start(out=ident[:], in_=ident_in[:])
            ztile = constp.tile([128, 512], f32, tag="ztile")
            nc.vector.memset(ztile[:], 0.0)

            mT_in = [None, m1T_in, m2T_in]

            for rep in range(cfg.reps):
                nc.sync.dma_start(out=hT[:, :], in_=x0T_in[:])

                for l in range(3):
                    ZR = 2 * NSP
                    for acc in (accA[l], accB[l]):
                        r = 0
                        while r < ZR:
                            n = min(512, ZR - r)
                            ap = acc[r:r + n, :].rearrange(
                                "(a p) d -> p a d", p=128)
                            zap = ztile[:, 0:n].rearrange(
                                "p (a d) -> p a d", d=128)
                            nc.sync.dma_start(out=ap, in_=zap)
                            r += n

                    for t in range(NT):
                        ps = mmp.tile([128, 128], f32, tag="mm")
                        nc.tensor.matmul(ps[:], hT[:, t * 128:(t + 1) * 128],
                                         Wt[l][:], start=True, stop=True)
                        gs = gsbp.tile([128, 128], f32, tag="gsb")
                        nc.vector.tensor_copy(gs[:], ps[:])
                        nc.sync.dma_start(out=gc[l][t * 128:(t + 1) * 128, :],
                                          in_=gs[:])

                    nc.gpsimd.collective_compute(
                        "AllGather", mybir.AluOpType.bypass,
                        replica_groups=[list(range(cfg.P))],
                        ins=[gc[l].ap().opt()], outs=[table[l].ap().opt()],
                    )

                    for (b, p0, p1, pieces) in tpl.chunks:
                        ni = p1 - p0
                        msg = msgp.tile([128, cfg.CH // 128, D], f32, tag="msg")
                        sidx = idxp.tile([128, cfg.CH // 16], i16, tag="sidx")
                        didx = idxp.tile([128, cfg.CH // 16], i16, tag="didx")
                        nc.sync.dma_start(
                            out=sidx[:, 0:ni // 16],
                            in_=srcw_in[:, p0 // 16:p1 // 16])
                        nc.sync.dma_start(
                            out=didx[:, 0:ni // 16],
                            in_=dstw_in[:, p0 // 16:p1 // 16])
                        nc.gpsimd.dma_gather(
                            msg[:, 0:ni // 128, :],
                            table[l][b * cfg.BROWS:(b + 1) * cfg.BROWS, :],
                            sidx[:, 0:ni // 16], ni, ni, D,
                            single_packet=False, queue_num=0)
                        for (gpar, s0, s1) in pieces:
                            acc = accA[l] if gpar == 0 else accB[l]
                            o0, o1 = (s0 - p0) // 128, (s1 - p0) // 128
                            nc.gpsimd.dma_scatter_add(
                                acc[:], msg[:, o0:o1, :],
                                didx[:, (s0 - p0) // 16:(s1 - p0) // 16],
                                s1 - s0, s1 - s0, D,
                                single_packet=False, queue_num=0)

                    for t in range(NT):
                        r0, r1 = t * 128, (t + 1) * 128
                        a0 = rbp.tile([128, 128], f32, tag="a0")
                        a1 = rbp.tile([128, 128], f32, tag="a1")
                        b0 = rbp.tile([128, 128], f32, tag="b0")
                        b1 = rbp.tile([128, 128], f32, tag="b1")
                        gt = rbp.tile([128, 128], f32, tag="gt")
                        nc.sync.dma_start(out=a0[:], in_=accA[l][r0:r1, :])
                        nc.sync.dma_start(out=a1[:],
                                          in_=accA[l][NSP + r0:NSP + r1, :])
                        nc.sync.dma_start(out=b0[:], in_=accB[l][r0:r1, :])
                        nc.sync.dma_start(out=b1[:],
                                          in_=accB[l][NSP + r0:NSP + r1, :])
                        nc.sync.dma_start(out=gt[:], in_=gc[l][r0:r1, :])
                        s = rbp.tile([128, 128], f32, tag="s")
                        nc.vector.tensor_add(s[:], a0[:], a1[:])
                        nc.vector.tensor_add(s[:], s[:], b0[:])
                        nc.vector.tensor_add(s[:], s[:], b1[:])
                        nc.vector.tensor_add(s[:], s[:], gt[:])
                        nc.vector.tensor_scalar_mul(s[:], s[:],
                                                    dinvc[:, t:t + 1])
                        nc.vector.tensor_add(s[:], s[:], bBt[l][:])
                        if l < 2:
                            nc.vector.tensor_scalar_max(s[:], s[:], 0.0)
                            tp = mmp.tile([128, 128], f32, tag="tr")
                            nc.tensor.transpose(tp[:], s[:], ident[:])
                            mt = rbp.tile([128, 128], f32, tag="mt")
                            nc.sync.dma_start(out=mt[:],
                                              in_=mT_in[l + 1][t, :, :])
                            nc.vector.tensor_mul(hT[:, r0:r1], tp[:], mt[:])
                        else:
                            if r0 < NS:
                                n = min(128, NS - r0)
                                nc.sync.dma_start(out=y_out[r0:r0 + n, :],
                                                  in_=s[0:n, :])
    nc.compile()
    return nc


def prep_inputs(cfg, features, edge_index, Ws, bs):
    import jax

    N, D, P, NS, NSP, NT = cfg.N, cfg.D, cfg.P, cfg.NS, cfg.NSP, cfg.NT
    src = np.asarray(edge_index[0], dtype=np.int64)
    dst = np.asarray(edge_index[1], dtype=np.int64)

    deg = np.zeros(N, np.float64)
    np.add.at(deg, dst, 1.0)
    deg += 1.0
    dinv = (1.0 / np.sqrt(deg)).astype(np.float32)

    # reproduce the reference's dropout masks exactly (threefry is
    # backend-deterministic; pin to CPU to avoid device detours)
    cpu = jax.devices("cpu")[0]
    with jax.default_device(cpu):
        dk = jax.random.key(1)
        keeps = []
        for i in range(3):
            k = jax.random.fold_in(dk, i)
            keeps.append(np.asarray(
                jax.random.bernoulli(k, 0.5, (N, D))).astype(np.float32))

    tpl, idx_pc = prep_edges(cfg, src, dst)

    feats = np.asarray(features, dtype=np.float32)
    ident = np.eye(128, dtype=np.float32)
    per_core = []
    for c in range(P):
        sh = slice(c * NS, (c + 1) * NS)
        dsh = dinv[sh]
        scale = (2.0 * dsh)[:, None]

        def padT(a):
            out = np.zeros((128, NSP), np.float32)
            out[:, :NS] = a.T
            return out

        x0T = padT(feats[sh] * keeps[0][sh] * scale)
        m1T = padT(keeps[1][sh] * scale)
        m2T = padT(keeps[2][sh] * scale)
        m1T = np.ascontiguousarray(m1T.reshape(128, NT, 128).transpose(1, 0, 2))
        m2T = np.ascontiguousarray(m2T.reshape(128, NT, 128).transpose(1, 0, 2))
        dpad = np.ones(NSP, np.float32)
        dpad[:NS] = dsh
        dinvc = np.ascontiguousarray(dpad.reshape(NT, 128).T)
        sw, dw = idx_pc[c]
        im = {"x0T": x0T, "m1T": m1T, "m2T": m2T, "dinvc": dinvc,
              "ident": ident, "srcw": sw, "dstw": dw}
        for i in range(3):
            im[f"W{i}"] = np.asarray(Ws[i], np.float32)
            im[f"bB{i}"] = np.tile(np.asarray(bs[i], np.float32)[None, :],
                                   (128, 1))
        per_core.append(im)
    return tpl, per_core


def kernel(features, edge_index, W0, b0, W1, b1, W2, b2):
    import concourse.bass_utils as bass_utils

    cfg = Cfg()
    tpl, per_core = prep_inputs(cfg, features, edge_index,
                                [W0, W1, W2], [b0, b1, b2])
    nc = build_nc(cfg, tpl)
    res = bass_utils.run_bass_kernel_spmd(nc, per_core,
                                          core_ids=list(range(cfg.P)))
    out = np.concatenate([res.results[c]["y"] for c in range(cfg.P)],
                         axis=0)[:cfg.N]
    return np.ascontiguousarray(out.astype(np.float32))
